# revision 8
# baseline (speedup 1.0000x reference)
"""Trainium2 Bass kernel for nn_Attention (sparse_attention, 8 NeuronCores).

Sharding: data-parallel over batch (4) x tensor-parallel over heads (2 groups
of 4 heads) = 8 cores. Each core computes attention for one batch and 4 heads
entirely in transposed (feature-major) layout, so no on-chip transposes are
needed. Wo is row-sharded; the two head-group partials per batch are summed on
the host during unsharding.
"""

import os
import sys

for _p in ("/opt/trn_rl_repo", "/root/.axon_site/_ro/trn_rl_repo"):
    if os.path.isdir(_p) and _p not in sys.path:
        sys.path.append(_p)

import numpy as np

B, N, DIM, H, DH = 4, 1024, 512, 8, 64
SCALE = DH**-0.5
HL = 4  # heads per core
HDL = HL * DH  # 256 head-dims per core
NCORES = 8
NJT = N // 128  # 8 key-tiles
NKT = DIM // 128  # 4 contraction tiles

_CACHE = {}


def _build(loop_iters=1):
    import concourse.tile as tile
    from concourse import bacc, mybir

    fp32 = mybir.dt.float32
    f32r = mybir.dt.float32r

    def r(ap):  # operands already declared float32r
        return ap
    Exp = mybir.ActivationFunctionType.Exp
    Identity = mybir.ActivationFunctionType.Identity
    add = mybir.AluOpType.add
    mult = mybir.AluOpType.mult

    nc = bacc.Bacc("TRN2", target_bir_lowering=False, debug=False, num_devices=NCORES)

    xT = nc.dram_tensor("xT", [128, NKT * N], f32r, kind="ExternalInput").ap()
    wq = nc.dram_tensor("wq", [128, NKT * HDL], f32r, kind="ExternalInput").ap()
    wk = nc.dram_tensor("wk", [128, NKT * HDL], f32r, kind="ExternalInput").ap()
    wv = nc.dram_tensor("wv", [128, NKT * HDL], f32r, kind="ExternalInput").ap()
    wg = nc.dram_tensor("wg", [128, NKT * HDL], f32r, kind="ExternalInput").ap()
    wo = nc.dram_tensor("wo", [128, 2 * DIM], f32r, kind="ExternalInput").ap()
    bgp = nc.dram_tensor("bg", [128, 2], fp32, kind="ExternalInput").ap()
    biasT = nc.dram_tensor(
        "biasT", [2, NJT, 128, 2 * N], fp32, kind="ExternalInput"
    ).ap()
    onesd = nc.dram_tensor("onesd", [128, 64], f32r, kind="ExternalInput").ap()
    outT = nc.dram_tensor("outT", [4, 128, N], fp32, kind="ExternalOutput").ap()

    from contextlib import ExitStack

    with tile.TileContext(nc) as tc, ExitStack() as stack:
        if loop_iters > 1:
            stack.enter_context(
                tc.For_i(0, loop_iters, 1, hint_engines=(mybir.EngineType.PE,))
            )
        with (
            tc.tile_pool(name="const", bufs=1) as cpool,
            tc.tile_pool(name="proj", bufs=1) as projpool,
            tc.tile_pool(name="bias", bufs=4) as biaspool,
            tc.tile_pool(name="etile", bufs=3) as epool,
            tc.tile_pool(name="work", bufs=3) as workpool,
            tc.tile_pool(name="psA", bufs=2, space="PSUM") as psA,
            tc.tile_pool(name="psB", bufs=2, space="PSUM") as psB,
        ):
            # ---- constants / weights in ----
            xT_sb = cpool.tile([128, NKT * N], f32r)
            nc.sync.dma_start(xT_sb[:], xT[:])
            wq_sb = cpool.tile([128, NKT * HDL], f32r, tag="wq")
            nc.sync.dma_start(wq_sb[:], wq[:])
            wk_sb = cpool.tile([128, NKT * HDL], f32r, tag="wk")
            nc.sync.dma_start(wk_sb[:], wk[:])
            wv_sb = cpool.tile([128, NKT * HDL], f32r, tag="wv")
            nc.sync.dma_start(wv_sb[:], wv[:])
            wg_sb = cpool.tile([128, NKT * HDL], f32r, tag="wg")
            nc.sync.dma_start(wg_sb[:], wg[:])
            wo_sb = cpool.tile([128, 2 * DIM], f32r, tag="wo")
            nc.sync.dma_start(wo_sb[:], wo[:])
            bg_sb = cpool.tile([128, 2], fp32, tag="bg")
            nc.sync.dma_start(bg_sb[:], bgp[:])
            ones_sb = cpool.tile([1, 64], f32r, tag="ones")
            nc.sync.dma_start(ones_sb[:], onesd[0:1, :])

            # ---- projections: qT/kT/gT = W.T @ x.T  (feature-major) ----
            qT_sb = [projpool.tile([128, N], f32r, tag=f"qT{m}", name=f"qT{m}") for m in range(2)]
            kT_sb = [projpool.tile([128, N], f32r, tag=f"kT{m}", name=f"kT{m}") for m in range(2)]
            gT_sb = [projpool.tile([128, N], fp32, tag=f"gT{m}", name=f"gT{m}") for m in range(2)]
            for w_sb, dst, biased in ((wq_sb, qT_sb, False), (wk_sb, kT_sb, False),
                                      (wg_sb, gT_sb, True)):
                for mt in range(2):
                    ps = psA.tile([128, N], fp32, tag="big")
                    for kt in range(NKT):
                        lhsT = w_sb[:, kt * HDL + mt * 128 : kt * HDL + mt * 128 + 128]
                        for ih in range(2):
                            nc.tensor.matmul(
                                ps[:, ih * 512 : ih * 512 + 512],
                                r(lhsT),
                                r(xT_sb[:, kt * N + ih * 512 : kt * N + ih * 512 + 512]),
                                start=(kt == 0),
                                stop=(kt == NKT - 1),
                            )
                    if biased:
                        nc.scalar.activation(
                            dst[mt][:], ps[:], Identity, bias=bg_sb[:, mt : mt + 1]
                        )
                    else:
                        nc.scalar.copy(dst[mt][:], ps[:])

            # ---- v natural [token, d] with appended ones column per head ----
            vhat_sb = [projpool.tile([128, HL * 65], f32r, tag=f"vh{j}", name=f"vh{j}") for j in range(NJT)]
            for jt in range(NJT):
                vv = vhat_sb[jt][:].rearrange("p (h c) -> p h c", h=HL)
                nc.sync.dma_start(vv[:, :, 64:65], onesd[:, 0:HL].rearrange("p (h o) -> p h o", o=1))
                ps2 = psB.tile([128, HDL], fp32, tag="uv")
                for kt in range(NKT):
                    nc.tensor.matmul(
                        ps2[:],
                        r(xT_sb[:, kt * N + jt * 128 : kt * N + jt * 128 + 128]),
                        r(wv_sb[:, kt * HDL : (kt + 1) * HDL]),
                        start=(kt == 0),
                        stop=(kt == NKT - 1),
                    )
                nc.scalar.copy(
                    vv[:, :, 0:64], ps2[:].rearrange("p (h c) -> p h c", h=HL)
                )

            # ---- attention per head-pair ----
            ug_sb = [workpool.tile([128, N], f32r, tag=f"ug{p}", name=f"ug{p}") for p in range(2)]
            for p in range(2):
                uv = [psB.tile([65, N], fp32, tag="uv", name=f"uv{p}_{i}") for i in range(2)]
                for jt in range(NJT):
                    bt = biaspool.tile([128, 2 * N], fp32, tag="bias")
                    nc.sync.dma_start(bt[:], biasT[p, jt])
                    st = [psA.tile([128, N], fp32, tag="big", name=f"st{jt}_{i}") for i in range(2)]
                    for hh in range(2):
                        lhsT = kT_sb[p][hh * 64 : hh * 64 + 64,
                                        jt * 128 : jt * 128 + 128]
                        for ih in range(2):
                            nc.tensor.matmul(
                                st[hh][:, ih * 512 : ih * 512 + 512],
                                r(lhsT),
                                r(qT_sb[p][hh * 64 : hh * 64 + 64,
                                           ih * 512 : ih * 512 + 512]),
                                start=True,
                                stop=True,
                            )
                    for hh in range(2):
                        nc.vector.tensor_tensor(
                            out=st[hh][:],
                            in0=st[hh][:],
                            in1=bt[:, hh * N : (hh + 1) * N],
                            op=add,
                        )
                        e = epool.tile([128, N], f32r, tag="e")
                        nc.scalar.activation(e[:], st[hh][:], Exp)
                        h = 2 * p + hh
                        for ih in range(2):
                            nc.tensor.matmul(
                                uv[hh][:, ih * 512 : ih * 512 + 512],
                                r(vhat_sb[jt][:, h * 65 : h * 65 + 65]),
                                r(e[:, ih * 512 : ih * 512 + 512]),
                                start=(jt == 0),
                                stop=(jt == NJT - 1),
                            )
                # epilogue: divide by softmax denom, multiply gates
                for hh in range(2):
                    rec = workpool.tile([1, N], f32r, tag="rec")
                    with nc.allow_low_precision(reason="f32r reciprocal feeds PE broadcast"):
                        nc.vector.reciprocal(rec[:], uv[hh][64:65, :])
                    bc = psA.tile([64, N], fp32, tag="big")
                    for ih in range(2):
                        nc.tensor.matmul(
                            bc[:, ih * 512 : ih * 512 + 512],
                            r(ones_sb[0:1, :]),
                            r(rec[0:1, ih * 512 : ih * 512 + 512]),
                            start=True,
                            stop=True,
                        )
                    gs = workpool.tile([64, N], fp32, tag="gs")
                    nc.vector.tensor_tensor(
                        out=gs[:],
                        in0=bc[:],
                        in1=gT_sb[p][hh * 64 : hh * 64 + 64, :],
                        op=mult,
                    )
                    nc.vector.tensor_tensor(
                        out=ug_sb[p][hh * 64 : hh * 64 + 64, :],
                        in0=uv[hh][0:64, :],
                        in1=gs[:],
                        op=mult,
                    )

            # ---- output projection: outT = Wo_loc.T-partial (row-shard) ----
            for mt in range(4):
                ps = psA.tile([128, N], fp32, tag="big")
                for p in range(2):
                    lhsT = wo_sb[:, p * DIM + mt * 128 : p * DIM + mt * 128 + 128]
                    for ih in range(2):
                        nc.tensor.matmul(
                            ps[:, ih * 512 : ih * 512 + 512],
                            r(lhsT),
                            r(ug_sb[p][:, ih * 512 : ih * 512 + 512]),
                            start=(p == 0),
                            stop=(p == 1),
                        )
                osb = workpool.tile([128, N], fp32, tag="osb")
                nc.scalar.copy(osb[:], ps[:])
                nc.sync.dma_start(outT[mt], osb[:])

    nc.compile()
    return nc


def _shard_inputs(x, attn_bias, Wq, Wkv, Wg, bg, Wo):
    """Build per-core input maps (host-side layout prep)."""

    def kmaj(w):  # [512, F] -> [128, 4*F] with contraction-tile-major columns
        f = w.shape[1]
        return np.ascontiguousarray(
            w.reshape(NKT, 128, f).transpose(1, 0, 2).reshape(128, NKT * f)
        )

    in_maps = []
    for d in range(NCORES):
        b, g = d // 2, d % 2
        cs = slice(g * HDL, (g + 1) * HDL)
        xTh = np.ascontiguousarray(x[b].T)  # [512, 1024]
        ab = attn_bias[b, g * HL : (g + 1) * HL]  # [4, 1024, 1024] (h, i, j)
        abT = ab.transpose(0, 2, 1).reshape(2, 2, NJT, 128, N)  # [pair, hh, jt, p, i]
        biasT = np.ascontiguousarray(abT.transpose(0, 2, 3, 1, 4)).reshape(
            2, NJT, 128, 2 * N
        )
        in_maps.append(
            {
                "xT": kmaj(xTh),
                "wq": kmaj(np.ascontiguousarray(Wq[:, cs]) * SCALE),
                "wk": kmaj(np.ascontiguousarray(Wkv[:, g * HDL : (g + 1) * HDL])),
                "wv": kmaj(
                    np.ascontiguousarray(Wkv[:, H * DH + g * HDL : H * DH + (g + 1) * HDL])
                ),
                "wg": kmaj(np.ascontiguousarray(Wg[:, cs])),
                "wo": np.ascontiguousarray(
                    Wo[cs, :].reshape(2, 128, DIM).transpose(1, 0, 2).reshape(128, 2 * DIM)
                ),
                "bg": np.ascontiguousarray(bg[cs].reshape(2, 128).T),
                "biasT": biasT,
                "onesd": np.ones((128, 64), np.float32),
            }
        )
    return in_maps


def _unshard(results, bo):
    out = np.empty((B, N, DIM), dtype=np.float32)
    for b in range(B):
        acc = results[2 * b]["outT"].astype(np.float32) + results[2 * b + 1][
            "outT"
        ].astype(np.float32)
        out[b] = acc.reshape(DIM, N).T + bo[None, :]
    return out


def kernel(x, mask, attn_bias, Wq, Wkv, Wg, bg, Wo, bo):
    """Full inputs in, full output out. mask is all-ones by construction."""
    from concourse.bass_utils import run_bass_kernel_spmd

    x = np.asarray(x, dtype=np.float32)
    attn_bias = np.asarray(attn_bias, dtype=np.float32)
    Wq = np.asarray(Wq, dtype=np.float32)
    Wkv = np.asarray(Wkv, dtype=np.float32)
    Wg = np.asarray(Wg, dtype=np.float32)
    bg = np.asarray(bg, dtype=np.float32)
    Wo = np.asarray(Wo, dtype=np.float32)
    bo = np.asarray(bo, dtype=np.float32)

    if "nc" not in _CACHE:
        _CACHE["nc"] = _build()
    in_maps = _shard_inputs(x, attn_bias, Wq, Wkv, Wg, bg, Wo)
    res = run_bass_kernel_spmd(_CACHE["nc"], in_maps, core_ids=list(range(NCORES)))
    return _unshard(res.results, bo)


# revision 9
# speedup vs baseline: 1.1884x; 1.1884x over previous
"""Trainium2 Bass kernel for nn_Attention (sparse_attention, 8 NeuronCores).

Sharding: data-parallel over batch (4) x tensor-parallel over heads (2 groups
of 4 heads) = 8 cores. Each core computes attention for one batch and 4 heads
entirely in transposed (feature-major) layout, so no on-chip transposes are
needed. Wo is row-sharded; the two head-group partials per batch are summed on
the host during unsharding.
"""

import os
import sys

for _p in ("/opt/trn_rl_repo", "/root/.axon_site/_ro/trn_rl_repo"):
    if os.path.isdir(_p) and _p not in sys.path:
        sys.path.append(_p)

import numpy as np

B, N, DIM, H, DH = 4, 1024, 512, 8, 64
SCALE = DH**-0.5
HL = 4  # heads per core
HDL = HL * DH  # 256 head-dims per core
NCORES = 8
NJT = N // 128  # 8 key-tiles
NKT = DIM // 128  # 4 contraction tiles

_CACHE = {}


def _build(loop_iters=1):
    import concourse.tile as tile
    from concourse import bacc, mybir

    fp32 = mybir.dt.float32
    f32r = mybir.dt.float32r
    bf16 = mybir.dt.bfloat16

    def r(ap):  # operands already declared float32r
        return ap
    Exp = mybir.ActivationFunctionType.Exp
    Identity = mybir.ActivationFunctionType.Identity
    add = mybir.AluOpType.add
    mult = mybir.AluOpType.mult

    nc = bacc.Bacc("TRN2", target_bir_lowering=False, debug=False, num_devices=NCORES)

    xT = nc.dram_tensor("xT", [128, NKT * N], f32r, kind="ExternalInput").ap()
    wq = nc.dram_tensor("wq", [128, NKT * HDL], f32r, kind="ExternalInput").ap()
    wk = nc.dram_tensor("wk", [128, NKT * HDL], f32r, kind="ExternalInput").ap()
    wv = nc.dram_tensor("wv", [128, NKT * HDL], f32r, kind="ExternalInput").ap()
    wg = nc.dram_tensor("wg", [128, NKT * HDL], f32r, kind="ExternalInput").ap()
    wo = nc.dram_tensor("wo", [128, 2 * DIM], f32r, kind="ExternalInput").ap()
    bgp = nc.dram_tensor("bg", [128, 2], fp32, kind="ExternalInput").ap()
    expB = nc.dram_tensor(
        "expB", [2, NJT, 128, 2 * N], bf16, kind="ExternalInput"
    ).ap()
    onesd = nc.dram_tensor("onesd", [128, 64], f32r, kind="ExternalInput").ap()
    onesb = nc.dram_tensor("onesb", [128, HL], bf16, kind="ExternalInput").ap()
    outT = nc.dram_tensor("outT", [4, 128, N], fp32, kind="ExternalOutput").ap()

    from contextlib import ExitStack

    with tile.TileContext(nc) as tc, ExitStack() as stack:
        if loop_iters > 1:
            stack.enter_context(
                tc.For_i(0, loop_iters, 1, hint_engines=(mybir.EngineType.PE,))
            )
        with (
            tc.tile_pool(name="const", bufs=1) as cpool,
            tc.tile_pool(name="proj", bufs=1) as projpool,
            tc.tile_pool(name="bias", bufs=4) as biaspool,
            tc.tile_pool(name="etile", bufs=3) as epool,
            tc.tile_pool(name="work", bufs=3) as workpool,
            tc.tile_pool(name="psA", bufs=2, space="PSUM") as psA,
            tc.tile_pool(name="psB", bufs=2, space="PSUM") as psB,
        ):
            # ---- constants / weights in ----
            xT_sb = cpool.tile([128, NKT * N], f32r)
            nc.sync.dma_start(xT_sb[:], xT[:])
            wq_sb = cpool.tile([128, NKT * HDL], f32r, tag="wq")
            nc.sync.dma_start(wq_sb[:], wq[:])
            wk_sb = cpool.tile([128, NKT * HDL], f32r, tag="wk")
            nc.sync.dma_start(wk_sb[:], wk[:])
            wv_sb = cpool.tile([128, NKT * HDL], f32r, tag="wv")
            nc.sync.dma_start(wv_sb[:], wv[:])
            wg_sb = cpool.tile([128, NKT * HDL], f32r, tag="wg")
            nc.sync.dma_start(wg_sb[:], wg[:])
            wo_sb = cpool.tile([128, 2 * DIM], f32r, tag="wo")
            nc.sync.dma_start(wo_sb[:], wo[:])
            bg_sb = cpool.tile([128, 2], fp32, tag="bg")
            nc.sync.dma_start(bg_sb[:], bgp[:])
            ones_sb = cpool.tile([1, 64], f32r, tag="ones")
            nc.sync.dma_start(ones_sb[:], onesd[0:1, :])

            # ---- projections: qT/kT/gT = W.T @ x.T  (feature-major) ----
            qT_sb = [projpool.tile([128, N], f32r, tag=f"qT{m}", name=f"qT{m}") for m in range(2)]
            kT_sb = [projpool.tile([128, N], f32r, tag=f"kT{m}", name=f"kT{m}") for m in range(2)]
            gT_sb = [projpool.tile([128, N], fp32, tag=f"gT{m}", name=f"gT{m}") for m in range(2)]
            for w_sb, dst, biased in ((wq_sb, qT_sb, False), (wk_sb, kT_sb, False),
                                      (wg_sb, gT_sb, True)):
                for mt in range(2):
                    ps = psA.tile([128, N], fp32, tag="big")
                    for kt in range(NKT):
                        lhsT = w_sb[:, kt * HDL + mt * 128 : kt * HDL + mt * 128 + 128]
                        for ih in range(2):
                            nc.tensor.matmul(
                                ps[:, ih * 512 : ih * 512 + 512],
                                r(lhsT),
                                r(xT_sb[:, kt * N + ih * 512 : kt * N + ih * 512 + 512]),
                                start=(kt == 0),
                                stop=(kt == NKT - 1),
                            )
                    if biased:
                        nc.scalar.activation(
                            dst[mt][:], ps[:], Identity, bias=bg_sb[:, mt : mt + 1]
                        )
                    else:
                        nc.scalar.copy(dst[mt][:], ps[:])

            # ---- v natural [token, d] with appended ones column per head ----
            vhat_sb = [projpool.tile([128, HL * 65], bf16, tag=f"vh{j}", name=f"vh{j}") for j in range(NJT)]
            for jt in range(NJT):
                vv = vhat_sb[jt][:].rearrange("p (h c) -> p h c", h=HL)
                nc.sync.dma_start(vv[:, :, 64:65], onesb[:, :].rearrange("p (h o) -> p h o", o=1))
                ps2 = psB.tile([128, HDL], fp32, tag="uv")
                for kt in range(NKT):
                    nc.tensor.matmul(
                        ps2[:],
                        r(xT_sb[:, kt * N + jt * 128 : kt * N + jt * 128 + 128]),
                        r(wv_sb[:, kt * HDL : (kt + 1) * HDL]),
                        start=(kt == 0),
                        stop=(kt == NKT - 1),
                    )
                nc.scalar.copy(
                    vv[:, :, 0:64], ps2[:].rearrange("p (h c) -> p h c", h=HL)
                )

            # ---- attention per head-pair ----
            ug_sb = [workpool.tile([128, N], f32r, tag=f"ug{p}", name=f"ug{p}") for p in range(2)]
            for p in range(2):
                uv = [psB.tile([65, N], fp32, tag="uv", name=f"uv{p}_{i}") for i in range(2)]
                for jt in range(NJT):
                    bt = biaspool.tile([128, 2 * N], bf16, tag="bias")
                    nc.sync.dma_start(bt[:], expB[p, jt])
                    st = [psA.tile([128, N], fp32, tag="big", name=f"st{jt}_{i}") for i in range(2)]
                    for hh in range(2):
                        lhsT = kT_sb[p][hh * 64 : hh * 64 + 64,
                                        jt * 128 : jt * 128 + 128]
                        for ih in range(2):
                            nc.tensor.matmul(
                                st[hh][:, ih * 512 : ih * 512 + 512],
                                r(lhsT),
                                r(qT_sb[p][hh * 64 : hh * 64 + 64,
                                           ih * 512 : ih * 512 + 512]),
                                start=True,
                                stop=True,
                            )
                    for hh in range(2):
                        e1 = epool.tile([128, N], bf16, tag="e1")
                        nc.scalar.activation(e1[:], st[hh][:], Exp)
                        e = epool.tile([128, N], bf16, tag="e")
                        nc.vector.tensor_tensor(
                            out=e[:],
                            in0=e1[:],
                            in1=bt[:, hh * N : (hh + 1) * N],
                            op=mult,
                        )
                        h = 2 * p + hh
                        for ih in range(2):
                            nc.tensor.matmul(
                                uv[hh][:, ih * 512 : ih * 512 + 512],
                                r(vhat_sb[jt][:, h * 65 : h * 65 + 65]),
                                r(e[:, ih * 512 : ih * 512 + 512]),
                                start=(jt == 0),
                                stop=(jt == NJT - 1),
                            )
                # epilogue: divide by softmax denom, multiply gates
                for hh in range(2):
                    rec = workpool.tile([1, N], f32r, tag="rec")
                    with nc.allow_low_precision(reason="f32r reciprocal feeds PE broadcast"):
                        nc.vector.reciprocal(rec[:], uv[hh][64:65, :])
                    bc = psA.tile([64, N], fp32, tag="big")
                    for ih in range(2):
                        nc.tensor.matmul(
                            bc[:, ih * 512 : ih * 512 + 512],
                            r(ones_sb[0:1, :]),
                            r(rec[0:1, ih * 512 : ih * 512 + 512]),
                            start=True,
                            stop=True,
                        )
                    gs = workpool.tile([64, N], fp32, tag="gs")
                    nc.vector.tensor_tensor(
                        out=gs[:],
                        in0=bc[:],
                        in1=gT_sb[p][hh * 64 : hh * 64 + 64, :],
                        op=mult,
                    )
                    nc.vector.tensor_tensor(
                        out=ug_sb[p][hh * 64 : hh * 64 + 64, :],
                        in0=uv[hh][0:64, :],
                        in1=gs[:],
                        op=mult,
                    )

            # ---- output projection: outT = Wo_loc.T-partial (row-shard) ----
            for mt in range(4):
                ps = psA.tile([128, N], fp32, tag="big")
                for p in range(2):
                    lhsT = wo_sb[:, p * DIM + mt * 128 : p * DIM + mt * 128 + 128]
                    for ih in range(2):
                        nc.tensor.matmul(
                            ps[:, ih * 512 : ih * 512 + 512],
                            r(lhsT),
                            r(ug_sb[p][:, ih * 512 : ih * 512 + 512]),
                            start=(p == 0),
                            stop=(p == 1),
                        )
                osb = workpool.tile([128, N], fp32, tag="osb")
                nc.vector.tensor_copy(osb[:], ps[:])
                nc.sync.dma_start(outT[mt], osb[:])

    nc.compile()
    return nc


def _shard_inputs(x, attn_bias, Wq, Wkv, Wg, bg, Wo):
    """Build per-core input maps (host-side layout prep)."""

    def kmaj(w):  # [512, F] -> [128, 4*F] with contraction-tile-major columns
        f = w.shape[1]
        return np.ascontiguousarray(
            w.reshape(NKT, 128, f).transpose(1, 0, 2).reshape(128, NKT * f)
        )

    in_maps = []
    for d in range(NCORES):
        b, g = d // 2, d % 2
        cs = slice(g * HDL, (g + 1) * HDL)
        xTh = np.ascontiguousarray(x[b].T)  # [512, 1024]
        ab = attn_bias[b, g * HL : (g + 1) * HL]  # [4, 1024, 1024] (h, i, j)
        abT = ab.transpose(0, 2, 1).reshape(2, 2, NJT, 128, N)  # [pair, hh, jt, p, i]
        import ml_dtypes
        expB = np.exp(abT.transpose(0, 2, 3, 1, 4)).astype(ml_dtypes.bfloat16).reshape(
            2, NJT, 128, 2 * N
        )
        in_maps.append(
            {
                "xT": kmaj(xTh),
                "wq": kmaj(np.ascontiguousarray(Wq[:, cs]) * SCALE),
                "wk": kmaj(np.ascontiguousarray(Wkv[:, g * HDL : (g + 1) * HDL])),
                "wv": kmaj(
                    np.ascontiguousarray(Wkv[:, H * DH + g * HDL : H * DH + (g + 1) * HDL])
                ),
                "wg": kmaj(np.ascontiguousarray(Wg[:, cs])),
                "wo": np.ascontiguousarray(
                    Wo[cs, :].reshape(2, 128, DIM).transpose(1, 0, 2).reshape(128, 2 * DIM)
                ),
                "bg": np.ascontiguousarray(bg[cs].reshape(2, 128).T),
                "expB": expB,
                "onesd": np.ones((128, 64), np.float32),
                "onesb": np.ones((128, HL), ml_dtypes.bfloat16),
            }
        )
    return in_maps


def _unshard(results, bo):
    out = np.empty((B, N, DIM), dtype=np.float32)
    for b in range(B):
        acc = results[2 * b]["outT"].astype(np.float32) + results[2 * b + 1][
            "outT"
        ].astype(np.float32)
        out[b] = acc.reshape(DIM, N).T + bo[None, :]
    return out


def kernel(x, mask, attn_bias, Wq, Wkv, Wg, bg, Wo, bo):
    """Full inputs in, full output out. mask is all-ones by construction."""
    from concourse.bass_utils import run_bass_kernel_spmd

    x = np.asarray(x, dtype=np.float32)
    attn_bias = np.asarray(attn_bias, dtype=np.float32)
    Wq = np.asarray(Wq, dtype=np.float32)
    Wkv = np.asarray(Wkv, dtype=np.float32)
    Wg = np.asarray(Wg, dtype=np.float32)
    bg = np.asarray(bg, dtype=np.float32)
    Wo = np.asarray(Wo, dtype=np.float32)
    bo = np.asarray(bo, dtype=np.float32)

    if "nc" not in _CACHE:
        _CACHE["nc"] = _build()
    in_maps = _shard_inputs(x, attn_bias, Wq, Wkv, Wg, bg, Wo)
    res = run_bass_kernel_spmd(_CACHE["nc"], in_maps, core_ids=list(range(NCORES)))
    return _unshard(res.results, bo)
